# revision 4
# baseline (speedup 1.0000x reference)
"""CircleLoss Trainium2 kernel (8-core SPMD), v2.2.

Math: for S = cosine-sim(enc, dec) [N,N], both loss directions reduce to
per-wrapped-diagonal logsumexps of one matrix:
    out = mean_{d=1..N-1} softplus(L[d] + lse_p)
    L[d]  = log sum_j exp(g(S[j,(j+d)%N])),  g(s) = GAMMA*(max(s,-M)^2 - M^2)
    lse_p = logsumexp_j h(S[j,j])  (exact, computed on host)
g in [-4, 60] so sum(exp(g)) fits f32 with no max-pass.

Device chain per element: w = sqrt(GAMMA)*s from a bf16 matmul (norms and
sqrt(GAMMA) folded into host-prepped operands), u = max(w,-2) as f16 —
drained from PSUM by DVE ts(max) on half the row-tiles and by ACT
Relu(w+2) + a fast 4x DVE shift(-2) on the other half (PSUM reads are 1x
on both engines; splitting balances them) — then v = u*u (DVE tt f16 2x,
row-wide), E = exp(v - 4) (ACT, row-wide, bf16 out).

Sharding: core r owns rows [1024r, 1024r+1024). Host pre-normalizes both
embeddings (f64) and ships bf16 transposed operands. Each core computes its
1024 x 8320 sheared slab (row-tile bj reads dec window cols shifted by
128*bj so wrapped diagonals align across tiles: element (p, y) of every
tile has diagonal d = y - p), accumulates 4 row-tiles per quad in bf16
(adds mostly on GpSimd), bounces the two quad stripes through DRAM with a
sheared re-read that turns diagonals into columns, and column-sums via
one-hot matmuls in PSUM. Host sums the 8 per-core [8192] partials and
finishes in float64.
"""

import numpy as np
import ml_dtypes

import concourse.bass as bass
import concourse.bacc as bacc
import concourse.mybir as mybir
from concourse.tile import TileContext
from concourse.bass_utils import run_bass_kernel_spmd

N = 8192
D = 128
P = 128
NCORES = 8
R = N // NCORES          # 1024 rows per core
NBJ = R // P             # 8 row-tiles per core
WIN = 9216               # dec window columns per core
W2 = 8320                # sheared slab width (y = x - 128*bj, d = y - p)
NWC = 16                 # 512-wide d-chunks in the output
M_M = 0.25
GAMMA = 64.0
SQG = 8.0                # sqrt(GAMMA), folded into enc operand on host
EPS = 1e-5

# per-bj wide groups over y: 5 x 1536 + 1 x 640
GROUPS = [(0, 1536), (1536, 1536), (3072, 1536), (4608, 1536),
          (6144, 1536), (7680, 640)]
ACT_DRAIN_ROWS = (1, 3, 5, 7)   # rows whose PSUM drain goes via ACT Relu

F32 = mybir.dt.float32
F16 = mybir.dt.float16
BF16 = mybir.dt.bfloat16

_CACHE = {}


def _build_program():
    nc = bacc.Bacc("TRN2", target_bir_lowering=False, debug=False,
                   num_devices=NCORES)
    encT = nc.dram_tensor("encT", [P, R], BF16, kind="ExternalInput")
    decT = nc.dram_tensor("decT", [P, WIN], BF16, kind="ExternalInput")
    acc_out = nc.dram_tensor("acc_out", [NWC, 512], F32, kind="ExternalOutput")
    stripes = nc.dram_tensor("stripes", [2, P, W2], BF16, kind="Internal")

    mx = mybir.AluOpType.max
    add = mybir.AluOpType.add
    AF = mybir.ActivationFunctionType

    with TileContext(nc) as tc:
        with (
            tc.tile_pool(name="persist", bufs=1) as persist,
            tc.tile_pool(name="mm", bufs=2, space="PSUM") as mmp,
            tc.tile_pool(name="apsum", bufs=1, space="PSUM") as apsum,
            tc.tile_pool(name="rpool", bufs=2) as rpool,
            tc.tile_pool(name="urow", bufs=2) as urow_pool,
            tc.tile_pool(name="vrow", bufs=2) as vrow_pool,
            tc.tile_pool(name="erow", bufs=2) as erow_pool,
            tc.tile_pool(name="erpool", bufs=4) as erpool,
        ):
            enc_sb = persist.tile([P, R], BF16)
            nc.sync.dma_start(out=enc_sb[:], in_=encT[:, :])
            dec_sb = persist.tile([P, WIN], BF16)
            nc.sync.dma_start(out=dec_sb[:], in_=decT[:, :])

            onehot = persist.tile([P, NWC * NWC], BF16)
            bias_m4 = persist.tile([P, 1], F32)
            nc.vector.memset(bias_m4[:], -4.0)
            bias_p2 = persist.tile([P, 1], F32)
            nc.vector.memset(bias_p2[:], 2.0)
            nc.gpsimd.memset(onehot[:], 0.0)
            for wc in range(NWC):
                nc.gpsimd.memset(onehot[:, wc * NWC + wc:wc * NWC + wc + 1],
                                 1.0)

            q0 = persist.tile([P, W2], BF16)
            q1 = persist.tile([P, W2], BF16)
            quads = [q0, q1]
            acc_ps = apsum.tile([NWC, 512], F32)
            nc.vector.memset(acc_ps[:], 0.0)
            acc_sb = persist.tile([NWC, 512], F32)

            # sheared reads eligible after stripe write of group g completes:
            # read rq covers stripe cols [1024rq, 1024rq+1150]
            reads_after = {0: [0], 1: [1], 2: [2, 3], 3: [4], 4: [5, 6],
                           5: [7]}

            for q in range(2):
                for bjl in range(4):
                    bj = 4 * q + bjl
                    act_drain = bj in ACT_DRAIN_ROWS
                    ur = urow_pool.tile([P, W2], F16, tag="ur")
                    for g, (y0, gw) in enumerate(GROUPS):
                        ps = mmp.tile([P, 1536], F32, tag="ps")
                        for c0 in range(0, gw, 512):
                            cw = min(512, gw - c0)
                            nc.tensor.matmul(
                                ps[:, c0:c0 + cw],
                                lhsT=enc_sb[:, bj * P:(bj + 1) * P],
                                rhs=dec_sb[:, 128 * bj + y0 + c0:
                                           128 * bj + y0 + c0 + cw],
                                start=True, stop=True)
                        if act_drain:
                            rt = rpool.tile([P, 1536], F16, tag="rt")
                            nc.scalar.activation(rt[:, 0:gw], ps[:, 0:gw],
                                                 AF.Relu,
                                                 bias=bias_p2[:, 0:1],
                                                 scale=1.0)
                            nc.vector.tensor_scalar(
                                out=ur[:, y0:y0 + gw], in0=rt[:, 0:gw],
                                scalar1=-2.0, scalar2=None, op0=add)
                        else:
                            nc.vector.tensor_scalar(
                                out=ur[:, y0:y0 + gw], in0=ps[:, 0:gw],
                                scalar1=-2.0, scalar2=None, op0=mx)
                    vr = vrow_pool.tile([P, W2], F16, tag="vr")
                    nc.vector.tensor_mul(vr[:], ur[:], ur[:])
                    if bjl == 0:
                        nc.scalar.activation(quads[q][:], vr[:], AF.Exp,
                                             bias=bias_m4[:, 0:1], scale=1.0)
                    else:
                        er_ = erow_pool.tile([P, W2], BF16, tag="erow")
                        nc.scalar.activation(er_[:], vr[:], AF.Exp,
                                             bias=bias_m4[:, 0:1], scale=1.0)
                        if bjl == 3 and q == 1:
                            nc.vector.tensor_tensor(out=quads[q][:],
                                                    in0=er_[:],
                                                    in1=quads[q][:], op=add)
                        else:
                            nc.gpsimd.tensor_tensor(out=quads[q][:],
                                                    in0=er_[:],
                                                    in1=quads[q][:], op=add)
                    if bjl == 3:
                        for g, (y0, gw) in enumerate(GROUPS):
                            nc.sync.dma_start(
                                out=stripes[q, :, y0:y0 + gw],
                                in_=quads[q][:, y0:y0 + gw])
                            for rq in reads_after[g]:
                                er = erpool.tile([P, 1024], BF16, tag="er")
                                nc.sync.dma_start(
                                    out=er[:],
                                    in_=bass.AP(tensor=stripes,
                                                offset=q * P * W2 + 1024 * rq,
                                                ap=[[W2 + 1, P], [1, 1024]]))
                                for h in range(2):
                                    wc = 2 * rq + h
                                    nc.tensor.matmul(
                                        acc_ps[:],
                                        lhsT=onehot[:, wc * NWC:(wc + 1) * NWC],
                                        rhs=er[:, h * 512:(h + 1) * 512],
                                        start=False, stop=False,
                                        skip_group_check=True)
            nc.scalar.copy(acc_sb[:], acc_ps[:])
            nc.sync.dma_start(out=acc_out[:, :], in_=acc_sb[:])
    nc.compile()
    return nc


def make_in_maps(enc: np.ndarray, dec: np.ndarray):
    """Host prep: normalize in f64, fold sqrt(GAMMA) into enc, transpose,
    cast bf16, build per-core window slices. Returns (in_maps, lse_p)."""
    e64 = enc.astype(np.float64)
    d64 = dec.astype(np.float64)
    en = np.sqrt((e64 * e64).sum(1, keepdims=True))
    dn = np.sqrt((d64 * d64).sum(1, keepdims=True))
    encn8 = (e64 / en * SQG).astype(ml_dtypes.bfloat16)
    decn = (d64 / dn).astype(ml_dtypes.bfloat16)

    s_jj = (e64 * d64).sum(1) / (en[:, 0] * dn[:, 0] + EPS)
    h = -np.maximum(1.0 + M_M - s_jj, 0.0) * (s_jj - (1.0 - M_M)) * GAMMA
    hm = h.max()
    lse_p = hm + np.log(np.exp(h - hm).sum())

    in_maps = []
    for r in range(NCORES):
        idx = (r * R + np.arange(WIN)) % N
        in_maps.append({
            "encT": np.ascontiguousarray(encn8[r * R:(r + 1) * R].T),
            "decT": np.ascontiguousarray(decn[idx].T),
        })
    return in_maps, lse_p


def kernel(encoder_output: np.ndarray, decoder_output: np.ndarray) -> np.ndarray:
    enc = np.ascontiguousarray(encoder_output, dtype=np.float32)
    dec = np.ascontiguousarray(decoder_output, dtype=np.float32)
    assert enc.shape == (N, D) and dec.shape == (N, D)

    if "nc" not in _CACHE:
        _CACHE["nc"] = _build_program()
    nc = _CACHE["nc"]

    in_maps, lse_p = make_in_maps(enc, dec)
    res = run_bass_kernel_spmd(nc, in_maps, core_ids=list(range(NCORES)))

    sum_exp = np.zeros(N, dtype=np.float64)
    for r in range(NCORES):
        acc = res.results[r]["acc_out"].astype(np.float64)      # [NWC, 512]
        sum_exp += acc.reshape(N)                               # d = 512*wc + f

    L = np.log(sum_exp[1:])
    x = L + lse_p
    out = np.mean(np.log1p(np.exp(-np.abs(x))) + np.maximum(x, 0.0))
    return np.float32(out)


# revision 6
# speedup vs baseline: 1.1452x; 1.1452x over previous
"""CircleLoss Trainium2 kernel (8-core SPMD), v2.3.

Math: for S = cosine-sim(enc, dec) [N,N], both loss directions reduce to
per-wrapped-diagonal logsumexps of one matrix:
    out = mean_{d=1..N-1} softplus(L[d] + lse_p)
    L[d]  = log sum_j exp(g(S[j,(j+d)%N])),  g(s) = GAMMA*(max(s,-M)^2 - M^2)
    lse_p = logsumexp_j h(S[j,j])  (exact, computed on host)
g in [-4, 60] so sum(exp(g)) fits f32 with no max-pass.

Device chain per element: w = sqrt(GAMMA)*s from a bf16 matmul (norms and
sqrt(GAMMA) folded into host-prepped operands), u = max(w,-2) as f16 —
drained from PSUM by DVE ts(max) on half the row-tiles and by ACT
Relu(w+2) + a fast 4x DVE shift(-2) on the other half (PSUM reads are 1x
on both engines; splitting balances them) — then v = u*u (DVE tt f16 2x,
row-wide), E = exp(v - 4) (ACT, row-wide, bf16 out).

Sharding: core r owns rows [1024r, 1024r+1024). Host pre-normalizes both
embeddings (f64) and ships bf16 transposed operands. Each core computes its
1024 x 8320 sheared slab (row-tile bj reads dec window cols shifted by
128*bj so wrapped diagonals align across tiles: element (p, y) of every
tile has diagonal d = y - p), accumulates 4 row-tiles per quad in bf16
(adds mostly on GpSimd), bounces the two quad stripes through DRAM with a
sheared re-read that turns diagonals into columns, and column-sums via
one-hot matmuls in PSUM. Host sums the 8 per-core [8192] partials and
finishes in float64.
"""

import numpy as np
import ml_dtypes

import concourse.bass as bass
import concourse.bacc as bacc
import concourse.mybir as mybir
from concourse.tile import TileContext
from concourse.bass_utils import run_bass_kernel_spmd

N = 8192
D = 128
P = 128
NCORES = 8
R = N // NCORES          # 1024 rows per core
NBJ = R // P             # 8 row-tiles per core
WIN = 9216               # dec window columns per core
W2 = 8320                # sheared slab width (y = x - 128*bj, d = y - p)
NWC = 16                 # 512-wide d-chunks in the output
M_M = 0.25
GAMMA = 64.0
SQG = 8.0                # sqrt(GAMMA), folded into enc operand on host
EPS = 1e-5

# per-bj wide groups over y: 5 x 1536 + 1 x 640
GROUPS = [(0, 1536), (1536, 1536), (3072, 1536), (4608, 1536),
          (6144, 1536), (7680, 640)]
ACT_DRAIN_ROWS = (1, 3, 5, 7)   # rows whose PSUM drain goes via ACT Relu

F32 = mybir.dt.float32
F16 = mybir.dt.float16
BF16 = mybir.dt.bfloat16

_CACHE = {}


def _build_program():
    nc = bacc.Bacc("TRN2", target_bir_lowering=False, debug=False,
                   num_devices=NCORES)
    encT = nc.dram_tensor("encT", [P, R], BF16, kind="ExternalInput")
    decT = nc.dram_tensor("decT", [P, WIN], BF16, kind="ExternalInput")
    acc_out = nc.dram_tensor("acc_out", [NWC, 512], F32, kind="ExternalOutput")
    stripes = nc.dram_tensor("stripes", [2, P, W2], BF16, kind="Internal")

    mx = mybir.AluOpType.max
    add = mybir.AluOpType.add
    AF = mybir.ActivationFunctionType

    with TileContext(nc) as tc:
        with (
            tc.tile_pool(name="persist", bufs=1) as persist,
            tc.tile_pool(name="mm", bufs=2, space="PSUM") as mmp,
            tc.tile_pool(name="apsum", bufs=1, space="PSUM") as apsum,
            tc.tile_pool(name="rpool", bufs=2) as rpool,
            tc.tile_pool(name="urow", bufs=2) as urow_pool,
            tc.tile_pool(name="vrow", bufs=2) as vrow_pool,
            tc.tile_pool(name="erow", bufs=2) as erow_pool,
            tc.tile_pool(name="erpool", bufs=4) as erpool,
        ):
            enc_sb = persist.tile([P, R], BF16)
            nc.sync.dma_start(out=enc_sb[:], in_=encT[:, :])
            dec_sb = persist.tile([P, WIN], BF16)
            nc.sync.dma_start(out=dec_sb[:], in_=decT[:, :])

            onehot = persist.tile([P, NWC * NWC], BF16)
            bias_m4 = persist.tile([P, 1], F32)
            nc.vector.memset(bias_m4[:], -4.0)
            bias_p2 = persist.tile([P, 1], F32)
            nc.vector.memset(bias_p2[:], 2.0)
            nc.gpsimd.memset(onehot[:], 0.0)
            for wc in range(NWC):
                nc.gpsimd.memset(onehot[:, wc * NWC + wc:wc * NWC + wc + 1],
                                 1.0)

            q0 = persist.tile([P, W2], BF16)
            q1 = persist.tile([P, W2], BF16)
            quads = [q0, q1]
            acc_ps = apsum.tile([NWC, 512], F32)
            nc.vector.memset(acc_ps[:], 0.0)
            acc_sb = persist.tile([NWC, 512], F32)

            # sheared reads eligible after stripe write of group g completes:
            # read rq covers stripe cols [1024rq, 1024rq+1150]
            reads_after = {0: [0], 1: [1], 2: [2, 3], 3: [4], 4: [5, 6],
                           5: [7]}

            for q in range(2):
                for bjl in range(4):
                    bj = 4 * q + bjl
                    act_drain = bj in ACT_DRAIN_ROWS
                    ur = urow_pool.tile([P, W2], F16, tag="ur")
                    vr = vrow_pool.tile([P, W2], F16, tag="vr")
                    er_ = None if bjl == 0 else \
                        erow_pool.tile([P, W2], BF16, tag="erow")
                    for g, (y0, gw) in enumerate(GROUPS):
                        ps = mmp.tile([P, 1536], F32, tag="ps")
                        for c0 in range(0, gw, 512):
                            cw = min(512, gw - c0)
                            nc.tensor.matmul(
                                ps[:, c0:c0 + cw],
                                lhsT=enc_sb[:, bj * P:(bj + 1) * P],
                                rhs=dec_sb[:, 128 * bj + y0 + c0:
                                           128 * bj + y0 + c0 + cw],
                                start=True, stop=True)
                        if act_drain:
                            rt = rpool.tile([P, 1536], F16, tag="rt")
                            nc.scalar.activation(rt[:, 0:gw], ps[:, 0:gw],
                                                 AF.Relu,
                                                 bias=bias_p2[:, 0:1],
                                                 scale=1.0)
                            nc.vector.tensor_scalar(
                                out=ur[:, y0:y0 + gw], in0=rt[:, 0:gw],
                                scalar1=-2.0, scalar2=None, op0=add)
                        else:
                            nc.vector.tensor_scalar(
                                out=ur[:, y0:y0 + gw], in0=ps[:, 0:gw],
                                scalar1=-2.0, scalar2=None, op0=mx)
                        nc.vector.tensor_mul(vr[:, y0:y0 + gw],
                                             ur[:, y0:y0 + gw],
                                             ur[:, y0:y0 + gw])
                        if y0 + gw in (4608, W2):
                            h0 = 0 if y0 + gw == 4608 else 4608
                            hw = (y0 + gw) - h0
                            dst = quads[q] if bjl == 0 else er_
                            nc.scalar.activation(dst[:, h0:h0 + hw],
                                                 vr[:, h0:h0 + hw], AF.Exp,
                                                 bias=bias_m4[:, 0:1],
                                                 scale=1.0)
                            if bjl != 0:
                                eng = nc.vector if (bjl == 3 and q == 1) \
                                    else nc.gpsimd
                                eng.tensor_tensor(
                                    out=quads[q][:, h0:h0 + hw],
                                    in0=er_[:, h0:h0 + hw],
                                    in1=quads[q][:, h0:h0 + hw], op=add)
                    if bjl == 3:
                        for g, (y0, gw) in enumerate(GROUPS):
                            nc.sync.dma_start(
                                out=stripes[q, :, y0:y0 + gw],
                                in_=quads[q][:, y0:y0 + gw])
                            for rq in reads_after[g]:
                                er = erpool.tile([P, 1024], BF16, tag="er")
                                nc.sync.dma_start(
                                    out=er[:],
                                    in_=bass.AP(tensor=stripes,
                                                offset=q * P * W2 + 1024 * rq,
                                                ap=[[W2 + 1, P], [1, 1024]]))
                                for h in range(2):
                                    wc = 2 * rq + h
                                    nc.tensor.matmul(
                                        acc_ps[:],
                                        lhsT=onehot[:, wc * NWC:(wc + 1) * NWC],
                                        rhs=er[:, h * 512:(h + 1) * 512],
                                        start=False, stop=False,
                                        skip_group_check=True)
            nc.scalar.copy(acc_sb[:], acc_ps[:])
            nc.sync.dma_start(out=acc_out[:, :], in_=acc_sb[:])
    nc.compile()
    return nc


def make_in_maps(enc: np.ndarray, dec: np.ndarray):
    """Host prep: normalize in f64, fold sqrt(GAMMA) into enc, transpose,
    cast bf16, build per-core window slices. Returns (in_maps, lse_p)."""
    e64 = enc.astype(np.float64)
    d64 = dec.astype(np.float64)
    en = np.sqrt((e64 * e64).sum(1, keepdims=True))
    dn = np.sqrt((d64 * d64).sum(1, keepdims=True))
    encn8 = (e64 / en * SQG).astype(ml_dtypes.bfloat16)
    decn = (d64 / dn).astype(ml_dtypes.bfloat16)

    s_jj = (e64 * d64).sum(1) / (en[:, 0] * dn[:, 0] + EPS)
    h = -np.maximum(1.0 + M_M - s_jj, 0.0) * (s_jj - (1.0 - M_M)) * GAMMA
    hm = h.max()
    lse_p = hm + np.log(np.exp(h - hm).sum())

    in_maps = []
    for r in range(NCORES):
        idx = (r * R + np.arange(WIN)) % N
        in_maps.append({
            "encT": np.ascontiguousarray(encn8[r * R:(r + 1) * R].T),
            "decT": np.ascontiguousarray(decn[idx].T),
        })
    return in_maps, lse_p


def kernel(encoder_output: np.ndarray, decoder_output: np.ndarray) -> np.ndarray:
    enc = np.ascontiguousarray(encoder_output, dtype=np.float32)
    dec = np.ascontiguousarray(decoder_output, dtype=np.float32)
    assert enc.shape == (N, D) and dec.shape == (N, D)

    if "nc" not in _CACHE:
        _CACHE["nc"] = _build_program()
    nc = _CACHE["nc"]

    in_maps, lse_p = make_in_maps(enc, dec)
    res = run_bass_kernel_spmd(nc, in_maps, core_ids=list(range(NCORES)))

    sum_exp = np.zeros(N, dtype=np.float64)
    for r in range(NCORES):
        acc = res.results[r]["acc_out"].astype(np.float64)      # [NWC, 512]
        sum_exp += acc.reshape(N)                               # d = 512*wc + f

    L = np.log(sum_exp[1:])
    x = L + lse_p
    out = np.mean(np.log1p(np.exp(-np.abs(x))) + np.maximum(x, 0.0))
    return np.float32(out)


# revision 8
# speedup vs baseline: 1.3747x; 1.2004x over previous
"""CircleLoss Trainium2 kernel (8-core SPMD), v2.4.

Math: for S = cosine-sim(enc, dec) [N,N], both loss directions reduce to
per-wrapped-diagonal logsumexps of one matrix:
    out = mean_{d=1..N-1} softplus(L[d] + lse_p)
    L[d]  = log sum_j exp(g(S[j,(j+d)%N])),  g(s) = GAMMA*(max(s,-M)^2 - M^2)
    lse_p = logsumexp_j h(S[j,j])  (exact, computed on host)
g in [-4, 60] so sum(exp(g)) fits f32 with no max-pass.

Device chain per element: w = sqrt(GAMMA)*s from a bf16 matmul (norms and
sqrt(GAMMA) folded into host-prepped operands), u = max(w,-2) as f16 —
drained from PSUM by DVE ts(max) on half the row-tiles and by ACT
Relu(w+2) + a fast 4x DVE shift(-2) on the other half (PSUM reads are 1x
on both engines; splitting balances them) — then v = u*u (DVE tt f16 2x,
row-wide), E = exp(v - 4) (ACT, row-wide, bf16 out).

Sharding: core r owns rows [1024r, 1024r+1024). Host pre-normalizes both
embeddings (f64) and ships bf16 transposed operands. Each core computes its
1024 x 8320 sheared slab (row-tile bj reads dec window cols shifted by
128*bj so wrapped diagonals align across tiles: element (p, y) of every
tile has diagonal d = y - p), accumulates 4 row-tiles per quad in bf16
(adds mostly on GpSimd), bounces the two quad stripes through DRAM with a
sheared re-read that turns diagonals into columns, and column-sums via
one-hot matmuls in PSUM. Host sums the 8 per-core [8192] partials and
finishes in float64.
"""

import numpy as np
import ml_dtypes

import concourse.bass as bass
import concourse.bacc as bacc
import concourse.mybir as mybir
from concourse.tile import TileContext
from concourse.bass_utils import run_bass_kernel_spmd

N = 8192
D = 128
P = 128
NCORES = 8
R = N // NCORES          # 1024 rows per core
NBJ = R // P             # 8 row-tiles per core
WIN = 9216               # dec window columns per core
W2 = 8320                # sheared slab width (y = x - 128*bj, d = y - p)
NWC = 16                 # 512-wide d-chunks in the output
M_M = 0.25
GAMMA = 64.0
SQG = 8.0                # sqrt(GAMMA), folded into enc operand on host
EPS = 1e-5

# per-bj wide groups over y: 5 x 1536 + 1 x 640
GROUPS = [(0, 1536), (1536, 1536), (3072, 1536), (4608, 1536),
          (6144, 1536), (7680, 640)]
ACT_DRAIN_ROWS = ()              # rows whose PSUM drain goes via ACT Relu

F32 = mybir.dt.float32
F16 = mybir.dt.float16
BF16 = mybir.dt.bfloat16

_CACHE = {}


def _build_program():
    nc = bacc.Bacc("TRN2", target_bir_lowering=False, debug=False,
                   num_devices=NCORES)
    encT = nc.dram_tensor("encT", [P, R], BF16, kind="ExternalInput")
    decT = nc.dram_tensor("decT", [P, WIN], BF16, kind="ExternalInput")
    acc_out = nc.dram_tensor("acc_out", [NWC, 512], F32, kind="ExternalOutput")
    stripes = nc.dram_tensor("stripes", [2, P, W2], BF16, kind="Internal")

    mx = mybir.AluOpType.max
    add = mybir.AluOpType.add
    AF = mybir.ActivationFunctionType

    with TileContext(nc) as tc:
        with (
            tc.tile_pool(name="persist", bufs=1) as persist,
            tc.tile_pool(name="mm", bufs=2, space="PSUM") as mmp,
            tc.tile_pool(name="apsum", bufs=1, space="PSUM") as apsum,
            tc.tile_pool(name="rpool", bufs=2) as rpool,
            tc.tile_pool(name="urow", bufs=2) as urow_pool,
            tc.tile_pool(name="vrow", bufs=2) as vrow_pool,
            tc.tile_pool(name="erow", bufs=2) as erow_pool,
            tc.tile_pool(name="erpool", bufs=4) as erpool,
        ):
            enc_sb = persist.tile([P, R], BF16)
            nc.sync.dma_start(out=enc_sb[:], in_=encT[:, :])
            dec_sb = persist.tile([P, WIN], BF16)
            nc.sync.dma_start(out=dec_sb[:], in_=decT[:, :])

            onehot = persist.tile([P, NWC * NWC], BF16)
            bias_m4 = persist.tile([P, 1], F32)
            nc.vector.memset(bias_m4[:], -4.0)
            bias_p2 = persist.tile([P, 1], F32)
            nc.vector.memset(bias_p2[:], 2.0)
            nc.gpsimd.memset(onehot[:], 0.0)
            for wc in range(NWC):
                nc.gpsimd.memset(onehot[:, wc * NWC + wc:wc * NWC + wc + 1],
                                 1.0)

            q0 = persist.tile([P, W2], BF16)
            q1 = persist.tile([P, W2], BF16)
            quads = [q0, q1]
            acc_ps = apsum.tile([NWC, 512], F32)
            nc.vector.memset(acc_ps[:], 0.0)
            acc_sb = persist.tile([NWC, 512], F32)

            # sheared reads eligible after stripe write of group g completes:
            # read rq covers stripe cols [1024rq, 1024rq+1150]
            reads_after = {0: [0], 1: [1], 2: [2, 3], 3: [4], 4: [5, 6],
                           5: [7]}

            for q in range(2):
                for bjl in range(4):
                    bj = 4 * q + bjl
                    act_drain = bj in ACT_DRAIN_ROWS
                    ur = urow_pool.tile([P, W2], F16, tag="ur")
                    vr = vrow_pool.tile([P, W2], F16, tag="vr")
                    er_ = None if bjl == 0 else \
                        erow_pool.tile([P, W2], BF16, tag="erow")
                    for g, (y0, gw) in enumerate(GROUPS):
                        ps = mmp.tile([P, 1536], F32, tag="ps")
                        for c0 in range(0, gw, 512):
                            cw = min(512, gw - c0)
                            nc.tensor.matmul(
                                ps[:, c0:c0 + cw],
                                lhsT=enc_sb[:, bj * P:(bj + 1) * P],
                                rhs=dec_sb[:, 128 * bj + y0 + c0:
                                           128 * bj + y0 + c0 + cw],
                                start=True, stop=True)
                        if act_drain:
                            rt = rpool.tile([P, 1536], F16, tag="rt")
                            nc.scalar.activation(rt[:, 0:gw], ps[:, 0:gw],
                                                 AF.Relu,
                                                 bias=bias_p2[:, 0:1],
                                                 scale=1.0)
                            nc.vector.tensor_scalar(
                                out=ur[:, y0:y0 + gw], in0=rt[:, 0:gw],
                                scalar1=-2.0, scalar2=None, op0=add)
                        else:
                            nc.vector.tensor_scalar(
                                out=ur[:, y0:y0 + gw], in0=ps[:, 0:gw],
                                scalar1=-2.0, scalar2=None, op0=mx)
                        if (bj + g) % 2 == 1:
                            nc.scalar.activation(vr[:, y0:y0 + gw],
                                                 ur[:, y0:y0 + gw], AF.Square)
                        else:
                            nc.vector.tensor_mul(vr[:, y0:y0 + gw],
                                                 ur[:, y0:y0 + gw],
                                                 ur[:, y0:y0 + gw])
                        if y0 + gw in (4608, W2):
                            h0 = 0 if y0 + gw == 4608 else 4608
                            hw = (y0 + gw) - h0
                            dst = quads[q] if bjl == 0 else er_
                            nc.scalar.activation(dst[:, h0:h0 + hw],
                                                 vr[:, h0:h0 + hw], AF.Exp,
                                                 bias=bias_m4[:, 0:1],
                                                 scale=1.0)
                            if bjl != 0:
                                eng = nc.gpsimd if (bjl == 2 and
                                                    (h0 == 0 or q == 0)) \
                                    else nc.vector
                                eng.tensor_tensor(
                                    out=quads[q][:, h0:h0 + hw],
                                    in0=er_[:, h0:h0 + hw],
                                    in1=quads[q][:, h0:h0 + hw], op=add)
                    if bjl == 3:
                        for g, (y0, gw) in enumerate(GROUPS):
                            nc.sync.dma_start(
                                out=stripes[q, :, y0:y0 + gw],
                                in_=quads[q][:, y0:y0 + gw])
                            for rq in reads_after[g]:
                                er = erpool.tile([P, 1024], BF16, tag="er")
                                nc.sync.dma_start(
                                    out=er[:],
                                    in_=bass.AP(tensor=stripes,
                                                offset=q * P * W2 + 1024 * rq,
                                                ap=[[W2 + 1, P], [1, 1024]]))
                                for h in range(2):
                                    wc = 2 * rq + h
                                    nc.tensor.matmul(
                                        acc_ps[:],
                                        lhsT=onehot[:, wc * NWC:(wc + 1) * NWC],
                                        rhs=er[:, h * 512:(h + 1) * 512],
                                        start=False, stop=False,
                                        skip_group_check=True)
            nc.scalar.copy(acc_sb[:], acc_ps[:])
            nc.sync.dma_start(out=acc_out[:, :], in_=acc_sb[:])
    nc.compile()
    return nc


def make_in_maps(enc: np.ndarray, dec: np.ndarray):
    """Host prep: normalize in f64, fold sqrt(GAMMA) into enc, transpose,
    cast bf16, build per-core window slices. Returns (in_maps, lse_p)."""
    e64 = enc.astype(np.float64)
    d64 = dec.astype(np.float64)
    en = np.sqrt((e64 * e64).sum(1, keepdims=True))
    dn = np.sqrt((d64 * d64).sum(1, keepdims=True))
    encn8 = (e64 / en * SQG).astype(ml_dtypes.bfloat16)
    decn = (d64 / dn).astype(ml_dtypes.bfloat16)

    s_jj = (e64 * d64).sum(1) / (en[:, 0] * dn[:, 0] + EPS)
    h = -np.maximum(1.0 + M_M - s_jj, 0.0) * (s_jj - (1.0 - M_M)) * GAMMA
    hm = h.max()
    lse_p = hm + np.log(np.exp(h - hm).sum())

    in_maps = []
    for r in range(NCORES):
        idx = (r * R + np.arange(WIN)) % N
        in_maps.append({
            "encT": np.ascontiguousarray(encn8[r * R:(r + 1) * R].T),
            "decT": np.ascontiguousarray(decn[idx].T),
        })
    return in_maps, lse_p


def kernel(encoder_output: np.ndarray, decoder_output: np.ndarray) -> np.ndarray:
    enc = np.ascontiguousarray(encoder_output, dtype=np.float32)
    dec = np.ascontiguousarray(decoder_output, dtype=np.float32)
    assert enc.shape == (N, D) and dec.shape == (N, D)

    if "nc" not in _CACHE:
        _CACHE["nc"] = _build_program()
    nc = _CACHE["nc"]

    in_maps, lse_p = make_in_maps(enc, dec)
    res = run_bass_kernel_spmd(nc, in_maps, core_ids=list(range(NCORES)))

    sum_exp = np.zeros(N, dtype=np.float64)
    for r in range(NCORES):
        acc = res.results[r]["acc_out"].astype(np.float64)      # [NWC, 512]
        sum_exp += acc.reshape(N)                               # d = 512*wc + f

    L = np.log(sum_exp[1:])
    x = L + lse_p
    out = np.mean(np.log1p(np.exp(-np.abs(x))) + np.maximum(x, 0.0))
    return np.float32(out)


# revision 11
# speedup vs baseline: 1.4044x; 1.0216x over previous
"""CircleLoss Trainium2 kernel (8-core SPMD), v2.4.

Math: for S = cosine-sim(enc, dec) [N,N], both loss directions reduce to
per-wrapped-diagonal logsumexps of one matrix:
    out = mean_{d=1..N-1} softplus(L[d] + lse_p)
    L[d]  = log sum_j exp(g(S[j,(j+d)%N])),  g(s) = GAMMA*(max(s,-M)^2 - M^2)
    lse_p = logsumexp_j h(S[j,j])  (exact, computed on host)
g in [-4, 60] so sum(exp(g)) fits f32 with no max-pass.

Device chain per element: w = sqrt(GAMMA)*s from a bf16 matmul (norms and
sqrt(GAMMA) folded into host-prepped operands), u = max(w,-2) as f16 —
drained from PSUM by DVE ts(max) on half the row-tiles and by ACT
Relu(w+2) + a fast 4x DVE shift(-2) on the other half (PSUM reads are 1x
on both engines; splitting balances them) — then v = u*u (DVE tt f16 2x,
row-wide), E = exp(v - 4) (ACT, row-wide, bf16 out).

Sharding: core r owns rows [1024r, 1024r+1024). Host pre-normalizes both
embeddings (f64) and ships bf16 transposed operands. Each core computes its
1024 x 8320 sheared slab (row-tile bj reads dec window cols shifted by
128*bj so wrapped diagonals align across tiles: element (p, y) of every
tile has diagonal d = y - p), accumulates 4 row-tiles per quad in bf16
(adds mostly on GpSimd), bounces the two quad stripes through DRAM with a
sheared re-read that turns diagonals into columns, and column-sums via
one-hot matmuls in PSUM. Host sums the 8 per-core [8192] partials and
finishes in float64.
"""

import numpy as np
import ml_dtypes

import concourse.bass as bass
import concourse.bacc as bacc
import concourse.mybir as mybir
from concourse.tile import TileContext
from concourse.bass_utils import run_bass_kernel_spmd

N = 8192
D = 128
P = 128
NCORES = 8
R = N // NCORES          # 1024 rows per core
NBJ = R // P             # 8 row-tiles per core
WIN = 9216               # dec window columns per core
W2 = 8320                # sheared slab width (y = x - 128*bj, d = y - p)
NWC = 16                 # 512-wide d-chunks in the output
M_M = 0.25
GAMMA = 64.0
SQG = 8.0                # sqrt(GAMMA), folded into enc operand on host
EPS = 1e-5

# per-bj wide groups over y: 5 x 1536 + 1 x 640
GROUPS = [(0, 1536), (1536, 1536), (3072, 1536), (4608, 1536),
          (6144, 1536), (7680, 640)]
ACT_DRAIN_ROWS = ()              # rows whose PSUM drain goes via ACT Relu

F32 = mybir.dt.float32
F16 = mybir.dt.float16
BF16 = mybir.dt.bfloat16

_CACHE = {}


def _build_program():
    nc = bacc.Bacc("TRN2", target_bir_lowering=False, debug=False,
                   num_devices=NCORES)
    encT = nc.dram_tensor("encT", [P, R], BF16, kind="ExternalInput")
    decT = nc.dram_tensor("decT", [P, WIN], BF16, kind="ExternalInput")
    acc_out = nc.dram_tensor("acc_out", [NWC, 512], F32, kind="ExternalOutput")
    stripes = nc.dram_tensor("stripes", [2, P, W2], BF16, kind="Internal")

    mx = mybir.AluOpType.max
    add = mybir.AluOpType.add
    AF = mybir.ActivationFunctionType

    with TileContext(nc) as tc:
        with (
            tc.tile_pool(name="persist", bufs=1) as persist,
            tc.tile_pool(name="mm", bufs=2, space="PSUM") as mmp,
            tc.tile_pool(name="apsum", bufs=1, space="PSUM") as apsum,
            tc.tile_pool(name="rpool", bufs=2) as rpool,
            tc.tile_pool(name="urow", bufs=2) as urow_pool,
            tc.tile_pool(name="vrow", bufs=2) as vrow_pool,
            tc.tile_pool(name="erow", bufs=2) as erow_pool,
            tc.tile_pool(name="erpool", bufs=4) as erpool,
        ):
            enc_sb = persist.tile([P, R], BF16)
            nc.sync.dma_start(out=enc_sb[:], in_=encT[:, :])
            dec_sb = persist.tile([P, WIN], BF16)
            nc.sync.dma_start(out=dec_sb[:], in_=decT[:, :])

            onehot = persist.tile([P, NWC * NWC], BF16)
            bias_m4 = persist.tile([P, 1], F32)
            nc.vector.memset(bias_m4[:], -4.0)
            bias_p2 = persist.tile([P, 1], F32)
            nc.vector.memset(bias_p2[:], 2.0)
            nc.gpsimd.memset(onehot[:], 0.0)
            for wc in range(NWC):
                nc.gpsimd.memset(onehot[:, wc * NWC + wc:wc * NWC + wc + 1],
                                 1.0)

            q0 = persist.tile([P, W2], BF16)
            q1 = persist.tile([P, W2], BF16)
            quads = [q0, q1]
            acc_ps = apsum.tile([NWC, 512], F32)
            nc.vector.memset(acc_ps[:], 0.0)
            acc_sb = persist.tile([NWC, 512], F32)

            # sheared reads eligible after stripe write of group g completes:
            # read rq covers stripe cols [1024rq, 1024rq+1150]
            reads_after = {0: [0], 1: [1], 2: [2, 3], 3: [4], 4: [5, 6],
                           5: [7]}

            for q in range(2):
                for bjl in range(4):
                    bj = 4 * q + bjl
                    act_drain = bj in ACT_DRAIN_ROWS
                    ur = urow_pool.tile([P, W2], F16, tag="ur")
                    vr = vrow_pool.tile([P, W2], F16, tag="vr")
                    er_ = None if bjl == 0 else \
                        erow_pool.tile([P, W2], BF16, tag="erow")
                    for g, (y0, gw) in enumerate(GROUPS):
                        ps = mmp.tile([P, 1536], F32, tag="ps")
                        for c0 in range(0, gw, 512):
                            cw = min(512, gw - c0)
                            nc.tensor.matmul(
                                ps[:, c0:c0 + cw],
                                lhsT=enc_sb[:, bj * P:(bj + 1) * P],
                                rhs=dec_sb[:, 128 * bj + y0 + c0:
                                           128 * bj + y0 + c0 + cw],
                                start=True, stop=True)
                        if act_drain:
                            rt = rpool.tile([P, 1536], F16, tag="rt")
                            nc.scalar.activation(rt[:, 0:gw], ps[:, 0:gw],
                                                 AF.Relu,
                                                 bias=bias_p2[:, 0:1],
                                                 scale=1.0)
                            nc.vector.tensor_scalar(
                                out=ur[:, y0:y0 + gw], in0=rt[:, 0:gw],
                                scalar1=-2.0, scalar2=None, op0=add)
                        else:
                            nc.vector.tensor_scalar(
                                out=ur[:, y0:y0 + gw], in0=ps[:, 0:gw],
                                scalar1=-2.0, scalar2=None, op0=mx)
                        if (bj + g) % 2 == 1:
                            nc.scalar.activation(vr[:, y0:y0 + gw],
                                                 ur[:, y0:y0 + gw], AF.Square)
                        else:
                            nc.vector.tensor_mul(vr[:, y0:y0 + gw],
                                                 ur[:, y0:y0 + gw],
                                                 ur[:, y0:y0 + gw])
                        if y0 + gw in (4608, W2):
                            h0 = 0 if y0 + gw == 4608 else 4608
                            hw = (y0 + gw) - h0
                            dst = quads[q] if bjl == 0 else er_
                            nc.scalar.activation(dst[:, h0:h0 + hw],
                                                 vr[:, h0:h0 + hw], AF.Exp,
                                                 bias=bias_m4[:, 0:1],
                                                 scale=1.0)
                            if bjl != 0:
                                eng = nc.gpsimd if (bjl == 2 and
                                                    (h0 == 0 or q == 0)) \
                                    else nc.vector
                                eng.tensor_tensor(
                                    out=quads[q][:, h0:h0 + hw],
                                    in0=er_[:, h0:h0 + hw],
                                    in1=quads[q][:, h0:h0 + hw], op=add)
                    if bjl == 3:
                        for g, (y0, gw) in enumerate(GROUPS):
                            nc.sync.dma_start(
                                out=stripes[q, :, y0:y0 + gw],
                                in_=quads[q][:, y0:y0 + gw])
                            for rq in reads_after[g]:
                                er = erpool.tile([P, 1024], BF16, tag="er")
                                nc.sync.dma_start(
                                    out=er[:],
                                    in_=bass.AP(tensor=stripes,
                                                offset=q * P * W2 + 1024 * rq,
                                                ap=[[W2 + 1, P], [1, 1024]]))
                                for h in range(2):
                                    wc = 2 * rq + h
                                    nc.tensor.matmul(
                                        acc_ps[:],
                                        lhsT=onehot[:, wc * NWC:(wc + 1) * NWC],
                                        rhs=er[:, h * 512:(h + 1) * 512],
                                        start=False, stop=False,
                                        skip_group_check=True)
            nc.scalar.copy(acc_sb[:], acc_ps[:])
            nc.sync.dma_start(out=acc_out[:, :], in_=acc_sb[:])
    nc.compile()
    return nc


def make_in_maps(enc: np.ndarray, dec: np.ndarray):
    """Host prep: normalize in f64, fold sqrt(GAMMA) into enc, transpose,
    cast bf16, build per-core window slices. Returns (in_maps, lse_p)."""
    e64 = enc.astype(np.float64)
    d64 = dec.astype(np.float64)
    en = np.sqrt((e64 * e64).sum(1, keepdims=True))
    dn = np.sqrt((d64 * d64).sum(1, keepdims=True))
    encn8 = (e64 / en * SQG).astype(ml_dtypes.bfloat16)
    decn = (d64 / dn).astype(ml_dtypes.bfloat16)

    s_jj = (e64 * d64).sum(1) / (en[:, 0] * dn[:, 0] + EPS)
    h = -np.maximum(1.0 + M_M - s_jj, 0.0) * (s_jj - (1.0 - M_M)) * GAMMA
    hm = h.max()
    lse_p = hm + np.log(np.exp(h - hm).sum())

    in_maps = []
    for r in range(NCORES):
        idx = (r * R + np.arange(WIN)) % N
        in_maps.append({
            "encT": np.ascontiguousarray(encn8[r * R:(r + 1) * R].T),
            "decT": np.ascontiguousarray(decn[idx].T),
        })
    return in_maps, lse_p


def kernel(encoder_output: np.ndarray, decoder_output: np.ndarray) -> np.ndarray:
    enc = np.ascontiguousarray(encoder_output, dtype=np.float32)
    dec = np.ascontiguousarray(decoder_output, dtype=np.float32)
    assert enc.shape == (N, D) and dec.shape == (N, D)

    if "nc" not in _CACHE:
        _CACHE["nc"] = _build_program()
    nc = _CACHE["nc"]

    in_maps, lse_p = make_in_maps(enc, dec)
    res = run_bass_kernel_spmd(nc, in_maps, core_ids=list(range(NCORES)))

    sum_exp = np.zeros(N, dtype=np.float64)
    for r in range(NCORES):
        acc = res.results[r]["acc_out"].astype(np.float64)      # [NWC, 512]
        sum_exp += acc.reshape(N)                               # d = 512*wc + f

    L = np.log(sum_exp[1:])
    x = L + lse_p
    out = np.mean(np.log1p(np.exp(-np.abs(x))) + np.maximum(x, 0.0))
    return np.float32(out)


# revision 13
# speedup vs baseline: 1.4292x; 1.0176x over previous
"""CircleLoss Trainium2 kernel (8-core SPMD), v2.6.

Math: for S = cosine-sim(enc, dec) [N,N], both loss directions reduce to
per-wrapped-diagonal logsumexps of one matrix:
    out = mean_{d=1..N-1} softplus(L[d] + lse_p)
    L[d]  = log sum_j exp(g(S[j,(j+d)%N])),  g(s) = GAMMA*(max(s,-M)^2 - M^2)
    lse_p = logsumexp_j h(S[j,j])  (exact, computed on host)
g in [-4, 60] so sum(exp(g)) fits f32 with no max-pass.

Device chain per element: w = sqrt(GAMMA)*s from a bf16 matmul (norms and
sqrt(GAMMA) folded into host-prepped operands), u = max(w,-2) as f16 —
drained from PSUM by DVE ts(max) on half the row-tiles and by ACT
Relu(w+2) + a fast 4x DVE shift(-2) on the other half (PSUM reads are 1x
on both engines; splitting balances them) — then v = u*u (DVE tt f16 2x,
row-wide), E = exp(v - 4) (ACT, row-wide, bf16 out).

Sharding: core r owns rows [1024r, 1024r+1024). Host pre-normalizes both
embeddings (f64) and ships bf16 transposed operands. Each core computes its
1024 x 8320 sheared slab (row-tile bj reads dec window cols shifted by
128*bj so wrapped diagonals align across tiles: element (p, y) of every
tile has diagonal d = y - p), accumulates 4 row-tiles per quad in bf16
(adds mostly on GpSimd), bounces the two quad stripes through DRAM with a
sheared re-read that turns diagonals into columns, and column-sums via
one-hot matmuls in PSUM. Host sums the 8 per-core [8192] partials and
finishes in float64.
"""

import numpy as np
import ml_dtypes

import concourse.bass as bass
import concourse.bacc as bacc
import concourse.mybir as mybir
from concourse.tile import TileContext
from concourse.bass_utils import run_bass_kernel_spmd

N = 8192
D = 128
P = 128
NCORES = 8
R = N // NCORES          # 1024 rows per core
NBJ = R // P             # 8 row-tiles per core
WIN = 9216               # dec window columns per core
W2 = 8320                # sheared slab width (y = x - 128*bj, d = y - p)
NWC = 16                 # 512-wide d-chunks in the output
M_M = 0.25
GAMMA = 64.0
SQG = 8.0                # sqrt(GAMMA), folded into enc operand on host
EPS = 1e-5

# per-bj wide groups over y: 5 x 1536 + 1 x 640
GROUPS = [(0, 1536), (1536, 1536), (3072, 1536), (4608, 1536),
          (6144, 1536), (7680, 640)]
ACT_DRAIN_ROWS = (2, 6)         # rows whose PSUM drain goes via ACT Relu

F32 = mybir.dt.float32
F16 = mybir.dt.float16
BF16 = mybir.dt.bfloat16

_CACHE = {}


def _build_program():
    nc = bacc.Bacc("TRN2", target_bir_lowering=False, debug=False,
                   num_devices=NCORES)
    encT = nc.dram_tensor("encT", [P, R], BF16, kind="ExternalInput")
    decT = nc.dram_tensor("decT", [P, WIN], BF16, kind="ExternalInput")
    acc_out = nc.dram_tensor("acc_out", [NWC, 512], F32, kind="ExternalOutput")
    stripes = nc.dram_tensor("stripes", [2, P, W2], BF16, kind="Internal")

    mx = mybir.AluOpType.max
    add = mybir.AluOpType.add
    AF = mybir.ActivationFunctionType

    with TileContext(nc) as tc:
        with (
            tc.tile_pool(name="persist", bufs=1) as persist,
            tc.tile_pool(name="mm", bufs=2, space="PSUM") as mmp,
            tc.tile_pool(name="apsum", bufs=1, space="PSUM") as apsum,
            tc.tile_pool(name="rpool", bufs=2) as rpool,
            tc.tile_pool(name="urow", bufs=2) as urow_pool,
            tc.tile_pool(name="vrow", bufs=2) as vrow_pool,
            tc.tile_pool(name="erow", bufs=3) as erow_pool,
            tc.tile_pool(name="erpool", bufs=4) as erpool,
        ):
            enc_sb = persist.tile([P, R], BF16)
            nc.sync.dma_start(out=enc_sb[:], in_=encT[:, :])
            dec_sb = persist.tile([P, WIN], BF16)
            for dk in range(3):
                nc.sync.dma_start(out=dec_sb[:, dk * 3072:(dk + 1) * 3072],
                                  in_=decT[:, dk * 3072:(dk + 1) * 3072])

            onehot = persist.tile([P, NWC * NWC], BF16)
            bias_m4 = persist.tile([P, 1], F32)
            nc.vector.memset(bias_m4[:], -4.0)
            bias_p2 = persist.tile([P, 1], F32)
            nc.vector.memset(bias_p2[:], 2.0)
            nc.gpsimd.memset(onehot[:], 0.0)
            for wc in range(NWC):
                nc.gpsimd.memset(onehot[:, wc * NWC + wc:wc * NWC + wc + 1],
                                 1.0)

            q0 = persist.tile([P, W2], BF16)
            q1 = persist.tile([P, W2], BF16)
            quads = [q0, q1]
            acc_ps = apsum.tile([NWC, 512], F32)
            nc.vector.memset(acc_ps[:], 0.0)
            acc_sb = persist.tile([NWC, 512], F32)

            # sheared reads eligible after stripe write of group g completes:
            # read rq covers stripe cols [1024rq, 1024rq+1150]
            reads_after = {0: [0], 1: [1], 2: [2, 3], 3: [4], 4: [5, 6],
                           5: [7]}

            for q in range(2):
                for bjl in range(4):
                    bj = 4 * q + bjl
                    act_drain = bj in ACT_DRAIN_ROWS
                    ur = urow_pool.tile([P, W2], F16, tag="ur")
                    vr = vrow_pool.tile([P, W2], F16, tag="vr")
                    er_ = None if bjl == 0 else \
                        erow_pool.tile([P, W2], BF16, tag="erow")
                    for g, (y0, gw) in enumerate(GROUPS):
                        ps = mmp.tile([P, 1536], F32, tag="ps")
                        for c0 in range(0, gw, 512):
                            cw = min(512, gw - c0)
                            nc.tensor.matmul(
                                ps[:, c0:c0 + cw],
                                lhsT=enc_sb[:, bj * P:(bj + 1) * P],
                                rhs=dec_sb[:, 128 * bj + y0 + c0:
                                           128 * bj + y0 + c0 + cw],
                                start=True, stop=True)
                        if act_drain:
                            rt = rpool.tile([P, 1536], F16, tag="rt")
                            nc.scalar.activation(rt[:, 0:gw], ps[:, 0:gw],
                                                 AF.Relu,
                                                 bias=bias_p2[:, 0:1],
                                                 scale=1.0)
                            nc.vector.tensor_scalar(
                                out=ur[:, y0:y0 + gw], in0=rt[:, 0:gw],
                                scalar1=-2.0, scalar2=None, op0=add)
                        else:
                            nc.vector.tensor_scalar(
                                out=ur[:, y0:y0 + gw], in0=ps[:, 0:gw],
                                scalar1=-2.0, scalar2=None, op0=mx)
                        nc.vector.tensor_mul(vr[:, y0:y0 + gw],
                                             ur[:, y0:y0 + gw],
                                             ur[:, y0:y0 + gw])
                        if y0 + gw in (4608, W2):
                            h0 = 0 if y0 + gw == 4608 else 4608
                            hw = (y0 + gw) - h0
                            dst = quads[q] if bjl == 0 else er_
                            nc.scalar.activation(dst[:, h0:h0 + hw],
                                                 vr[:, h0:h0 + hw], AF.Exp,
                                                 bias=bias_m4[:, 0:1],
                                                 scale=1.0)
                            if bjl != 0:
                                nc.vector.tensor_tensor(
                                    out=quads[q][:, h0:h0 + hw],
                                    in0=er_[:, h0:h0 + hw],
                                    in1=quads[q][:, h0:h0 + hw], op=add)
                    if bjl == 3:
                        for g, (y0, gw) in enumerate(GROUPS):
                            nc.sync.dma_start(
                                out=stripes[q, :, y0:y0 + gw],
                                in_=quads[q][:, y0:y0 + gw])
                            for rq in reads_after[g]:
                                er = erpool.tile([P, 1024], BF16, tag="er")
                                nc.sync.dma_start(
                                    out=er[:],
                                    in_=bass.AP(tensor=stripes,
                                                offset=q * P * W2 + 1024 * rq,
                                                ap=[[W2 + 1, P], [1, 1024]]))
                                for h in range(2):
                                    wc = 2 * rq + h
                                    nc.tensor.matmul(
                                        acc_ps[:],
                                        lhsT=onehot[:, wc * NWC:(wc + 1) * NWC],
                                        rhs=er[:, h * 512:(h + 1) * 512],
                                        start=False, stop=False,
                                        skip_group_check=True)
            nc.scalar.copy(acc_sb[:], acc_ps[:])
            nc.sync.dma_start(out=acc_out[:, :], in_=acc_sb[:])
    nc.compile()
    return nc


def make_in_maps(enc: np.ndarray, dec: np.ndarray):
    """Host prep: normalize in f64, fold sqrt(GAMMA) into enc, transpose,
    cast bf16, build per-core window slices. Returns (in_maps, lse_p)."""
    e64 = enc.astype(np.float64)
    d64 = dec.astype(np.float64)
    en = np.sqrt((e64 * e64).sum(1, keepdims=True))
    dn = np.sqrt((d64 * d64).sum(1, keepdims=True))
    encn8 = (e64 / en * SQG).astype(ml_dtypes.bfloat16)
    decn = (d64 / dn).astype(ml_dtypes.bfloat16)

    s_jj = (e64 * d64).sum(1) / (en[:, 0] * dn[:, 0] + EPS)
    h = -np.maximum(1.0 + M_M - s_jj, 0.0) * (s_jj - (1.0 - M_M)) * GAMMA
    hm = h.max()
    lse_p = hm + np.log(np.exp(h - hm).sum())

    in_maps = []
    for r in range(NCORES):
        idx = (r * R + np.arange(WIN)) % N
        in_maps.append({
            "encT": np.ascontiguousarray(encn8[r * R:(r + 1) * R].T),
            "decT": np.ascontiguousarray(decn[idx].T),
        })
    return in_maps, lse_p


def kernel(encoder_output: np.ndarray, decoder_output: np.ndarray) -> np.ndarray:
    enc = np.ascontiguousarray(encoder_output, dtype=np.float32)
    dec = np.ascontiguousarray(decoder_output, dtype=np.float32)
    assert enc.shape == (N, D) and dec.shape == (N, D)

    if "nc" not in _CACHE:
        _CACHE["nc"] = _build_program()
    nc = _CACHE["nc"]

    in_maps, lse_p = make_in_maps(enc, dec)
    res = run_bass_kernel_spmd(nc, in_maps, core_ids=list(range(NCORES)))

    sum_exp = np.zeros(N, dtype=np.float64)
    for r in range(NCORES):
        acc = res.results[r]["acc_out"].astype(np.float64)      # [NWC, 512]
        sum_exp += acc.reshape(N)                               # d = 512*wc + f

    L = np.log(sum_exp[1:])
    x = L + lse_p
    out = np.mean(np.log1p(np.exp(-np.abs(x))) + np.maximum(x, 0.0))
    return np.float32(out)


# revision 14
# speedup vs baseline: 1.5573x; 1.0897x over previous
"""CircleLoss Trainium2 kernel (8-core SPMD), v2.7.

Math: for S = cosine-sim(enc, dec) [N,N], both loss directions reduce to
per-wrapped-diagonal logsumexps of one matrix:
    out = mean_{d=1..N-1} softplus(L[d] + lse_p)
    L[d]  = log sum_j exp(g(S[j,(j+d)%N])),  g(s) = GAMMA*(max(s,-M)^2 - M^2)
    lse_p = logsumexp_j h(S[j,j])  (exact, computed on host)
g in [-4, 60] so sum(exp(g)) fits f32 with no max-pass.

Device chain per element: w = sqrt(GAMMA)*s from a bf16 matmul (norms and
sqrt(GAMMA) folded into host-prepped operands), u = max(w,-2) as f16 —
drained from PSUM by DVE ts(max) on half the row-tiles and by ACT
Relu(w+2) + a fast 4x DVE shift(-2) on the other half (PSUM reads are 1x
on both engines; splitting balances them) — then v = u*u (DVE tt f16 2x,
row-wide), E = exp(v - 4) (ACT, row-wide, bf16 out).

Sharding: core r owns rows [1024r, 1024r+1024). Host pre-normalizes both
embeddings (f64) and ships bf16 transposed operands. Each core computes its
1024 x 8320 sheared slab (row-tile bj reads dec window cols shifted by
128*bj so wrapped diagonals align across tiles: element (p, y) of every
tile has diagonal d = y - p), accumulates 4 row-tiles per quad in bf16
(adds mostly on GpSimd), bounces the two quad stripes through DRAM with a
sheared re-read that turns diagonals into columns, and column-sums via
one-hot matmuls in PSUM. Host sums the 8 per-core [8192] partials and
finishes in float64.
"""

import numpy as np
import ml_dtypes

import concourse.bass as bass
import concourse.bacc as bacc
import concourse.mybir as mybir
from concourse.tile import TileContext
from concourse.bass_utils import run_bass_kernel_spmd

N = 8192
D = 128
P = 128
NCORES = 8
R = N // NCORES          # 1024 rows per core
NBJ = R // P             # 8 row-tiles per core
WIN = 9216               # dec window columns per core
W2 = 8320                # sheared slab width (y = x - 128*bj, d = y - p)
NWC = 16                 # 512-wide d-chunks in the output
M_M = 0.25
GAMMA = 64.0
SQG = 8.0                # sqrt(GAMMA), folded into enc operand on host
EPS = 1e-5

# per-bj wide groups over y: 5 x 1536 + 1 x 640
GROUPS = [(0, 1536), (1536, 1536), (3072, 1536), (4608, 1536),
          (6144, 1536), (7680, 640)]
ACT_DRAIN_ROWS = (2, 6)         # rows whose PSUM drain goes via ACT Relu

F32 = mybir.dt.float32
F16 = mybir.dt.float16
BF16 = mybir.dt.bfloat16

_CACHE = {}


def _build_program():
    nc = bacc.Bacc("TRN2", target_bir_lowering=False, debug=False,
                   num_devices=NCORES)
    encT = nc.dram_tensor("encT", [P, R], BF16, kind="ExternalInput")
    decT = nc.dram_tensor("decT", [P, WIN], BF16, kind="ExternalInput")
    acc_out = nc.dram_tensor("acc_out", [NWC, 512], F32, kind="ExternalOutput")
    stripes = nc.dram_tensor("stripes", [2, P, W2], BF16, kind="Internal")

    mx = mybir.AluOpType.max
    add = mybir.AluOpType.add
    AF = mybir.ActivationFunctionType

    with TileContext(nc) as tc:
        with (
            tc.tile_pool(name="persist", bufs=1) as persist,
            tc.tile_pool(name="mm", bufs=2, space="PSUM") as mmp,
            tc.tile_pool(name="apsum", bufs=1, space="PSUM") as apsum,
            tc.tile_pool(name="rpool", bufs=2) as rpool,
            tc.tile_pool(name="urow", bufs=2) as urow_pool,
            tc.tile_pool(name="vrow", bufs=2) as vrow_pool,
            tc.tile_pool(name="erow", bufs=3) as erow_pool,
            tc.tile_pool(name="erpool", bufs=4) as erpool,
        ):
            enc_sb = persist.tile([P, R], BF16)
            nc.sync.dma_start(out=enc_sb[:], in_=encT[:, :])
            dec_sb = persist.tile([P, WIN], BF16)
            for dk in range(3):
                nc.sync.dma_start(out=dec_sb[:, dk * 3072:(dk + 1) * 3072],
                                  in_=decT[:, dk * 3072:(dk + 1) * 3072])

            onehot = persist.tile([P, NWC * NWC], BF16)
            bias_m4 = persist.tile([P, 1], F32)
            nc.vector.memset(bias_m4[:], -4.0)
            bias_p2 = persist.tile([P, 1], F32)
            nc.vector.memset(bias_p2[:], 2.0)
            nc.gpsimd.memset(onehot[:], 0.0)
            for wc in range(NWC):
                nc.gpsimd.memset(onehot[:, wc * NWC + wc:wc * NWC + wc + 1],
                                 1.0)

            q0 = persist.tile([P, W2], BF16)
            q1 = persist.tile([P, W2], BF16)
            quads = [q0, q1]
            acc_ps = apsum.tile([NWC, 512], F32)
            nc.vector.memset(acc_ps[:], 0.0)
            acc_sb = persist.tile([NWC, 512], F32)

            # sheared reads eligible after stripe write of group g completes:
            # read rq covers stripe cols [1024rq, 1024rq+1150]
            reads_after = {0: [0], 1: [1], 2: [2, 3], 3: [4], 4: [5, 6],
                           5: [7]}

            for q in range(2):
                for bjl in range(4):
                    bj = 4 * q + bjl
                    act_drain = bj in ACT_DRAIN_ROWS
                    ur = urow_pool.tile([P, W2], F16, tag="ur")
                    vr = vrow_pool.tile([P, W2], F16, tag="vr")
                    er_ = None if bjl == 0 else \
                        erow_pool.tile([P, W2], BF16, tag="erow")
                    for g, (y0, gw) in enumerate(GROUPS):
                        ps = mmp.tile([P, 1536], F32, tag="ps")
                        for c0 in range(0, gw, 512):
                            cw = min(512, gw - c0)
                            nc.tensor.matmul(
                                ps[:, c0:c0 + cw],
                                lhsT=enc_sb[:, bj * P:(bj + 1) * P],
                                rhs=dec_sb[:, 128 * bj + y0 + c0:
                                           128 * bj + y0 + c0 + cw],
                                start=True, stop=True)
                        if act_drain:
                            rt = rpool.tile([P, 1536], F16, tag="rt")
                            nc.scalar.activation(rt[:, 0:gw], ps[:, 0:gw],
                                                 AF.Relu,
                                                 bias=bias_p2[:, 0:1],
                                                 scale=1.0)
                            nc.vector.tensor_scalar(
                                out=ur[:, y0:y0 + gw], in0=rt[:, 0:gw],
                                scalar1=-2.0, scalar2=None, op0=add)
                        else:
                            nc.vector.tensor_scalar(
                                out=ur[:, y0:y0 + gw], in0=ps[:, 0:gw],
                                scalar1=-2.0, scalar2=None, op0=mx)
                        nc.vector.tensor_mul(vr[:, y0:y0 + gw],
                                             ur[:, y0:y0 + gw],
                                             ur[:, y0:y0 + gw])
                        if bjl == 3:
                            nc.scalar.activation(er_[:, y0:y0 + gw],
                                                 vr[:, y0:y0 + gw], AF.Exp,
                                                 bias=bias_m4[:, 0:1],
                                                 scale=1.0)
                            nc.vector.tensor_tensor(
                                out=quads[q][:, y0:y0 + gw],
                                in0=er_[:, y0:y0 + gw],
                                in1=quads[q][:, y0:y0 + gw], op=add)
                            nc.sync.dma_start(
                                out=stripes[q, :, y0:y0 + gw],
                                in_=quads[q][:, y0:y0 + gw])
                            for rq in reads_after[g]:
                                er = erpool.tile([P, 1024], BF16, tag="er")
                                nc.sync.dma_start(
                                    out=er[:],
                                    in_=bass.AP(tensor=stripes,
                                                offset=q * P * W2 + 1024 * rq,
                                                ap=[[W2 + 1, P], [1, 1024]]))
                                for h in range(2):
                                    wc = 2 * rq + h
                                    nc.tensor.matmul(
                                        acc_ps[:],
                                        lhsT=onehot[:, wc * NWC:(wc + 1) * NWC],
                                        rhs=er[:, h * 512:(h + 1) * 512],
                                        start=False, stop=False,
                                        skip_group_check=True)
                        elif y0 + gw in (4608, W2):
                            h0 = 0 if y0 + gw == 4608 else 4608
                            hw = (y0 + gw) - h0
                            dst = quads[q] if bjl == 0 else er_
                            nc.scalar.activation(dst[:, h0:h0 + hw],
                                                 vr[:, h0:h0 + hw], AF.Exp,
                                                 bias=bias_m4[:, 0:1],
                                                 scale=1.0)
                            if bjl != 0:
                                nc.vector.tensor_tensor(
                                    out=quads[q][:, h0:h0 + hw],
                                    in0=er_[:, h0:h0 + hw],
                                    in1=quads[q][:, h0:h0 + hw], op=add)
            nc.scalar.copy(acc_sb[:], acc_ps[:])
            nc.sync.dma_start(out=acc_out[:, :], in_=acc_sb[:])
    nc.compile()
    return nc


def make_in_maps(enc: np.ndarray, dec: np.ndarray):
    """Host prep: normalize in f64, fold sqrt(GAMMA) into enc, transpose,
    cast bf16, build per-core window slices. Returns (in_maps, lse_p)."""
    e64 = enc.astype(np.float64)
    d64 = dec.astype(np.float64)
    en = np.sqrt((e64 * e64).sum(1, keepdims=True))
    dn = np.sqrt((d64 * d64).sum(1, keepdims=True))
    encn8 = (e64 / en * SQG).astype(ml_dtypes.bfloat16)
    decn = (d64 / dn).astype(ml_dtypes.bfloat16)

    s_jj = (e64 * d64).sum(1) / (en[:, 0] * dn[:, 0] + EPS)
    h = -np.maximum(1.0 + M_M - s_jj, 0.0) * (s_jj - (1.0 - M_M)) * GAMMA
    hm = h.max()
    lse_p = hm + np.log(np.exp(h - hm).sum())

    in_maps = []
    for r in range(NCORES):
        idx = (r * R + np.arange(WIN)) % N
        in_maps.append({
            "encT": np.ascontiguousarray(encn8[r * R:(r + 1) * R].T),
            "decT": np.ascontiguousarray(decn[idx].T),
        })
    return in_maps, lse_p


def kernel(encoder_output: np.ndarray, decoder_output: np.ndarray) -> np.ndarray:
    enc = np.ascontiguousarray(encoder_output, dtype=np.float32)
    dec = np.ascontiguousarray(decoder_output, dtype=np.float32)
    assert enc.shape == (N, D) and dec.shape == (N, D)

    if "nc" not in _CACHE:
        _CACHE["nc"] = _build_program()
    nc = _CACHE["nc"]

    in_maps, lse_p = make_in_maps(enc, dec)
    res = run_bass_kernel_spmd(nc, in_maps, core_ids=list(range(NCORES)))

    sum_exp = np.zeros(N, dtype=np.float64)
    for r in range(NCORES):
        acc = res.results[r]["acc_out"].astype(np.float64)      # [NWC, 512]
        sum_exp += acc.reshape(N)                               # d = 512*wc + f

    L = np.log(sum_exp[1:])
    x = L + lse_p
    out = np.mean(np.log1p(np.exp(-np.abs(x))) + np.maximum(x, 0.0))
    return np.float32(out)
